# revision 15
# baseline (speedup 1.0000x reference)
"""Trainium2 Bass kernel for nn_Attention_78537771975200.

Data-parallel over bs*N = 16 object tracks -> 2 tracks per NeuronCore x 8 cores.

Per track (T=12, ch=128, hw=576):
  x_att  = L2-normalize(x) over channels
  E_a    = x_att[a+1]^T x_att[a]          (a = 0..10)   [n(query) x m(key)]
  A_a    = softmax(E_a * 128^-0.5 / temp) over m
  V_t    = concat(Wv[32:] @ x_t + bv[32:], posenc)      [114 x 576]
  out[t',   0:114] = V_{t'+3}
  out[t', 114:228] = P1_{t'+2},  P1_a = V_a A_a^T
  out[t', 228:342] = P2_{t'+2},  P2_a = P1_{a-1} A_a^T
  out[t', 342:456] = P3_{t'+2},  P3_a = P2_{a-1} A_a^T
(chain reuse: 29 products/track instead of reference's 54)

Softmax skips max-subtraction: |E*scale| <= 128^-0.5 (Cauchy-Schwarz on unit
vectors), so exp never overflows.
"""

import sys

sys.path.insert(0, "/opt/trn_rl_repo")

import numpy as np

from concourse import bass, bacc, mybir
from concourse import tile as tile_mod
from concourse.bass_utils import run_bass_kernel_spmd

# Route every ACT function to natural_log_exp_and_others (covers exp/ln/
# square/identity/copy) so the kernel needs exactly one ACT table load
# instead of ping-ponging sets (~2.7us per reload).
_orig_get_tables = bacc.get_activation_tables

def _single_set_tables(arch):
    t = _orig_get_tables(arch)
    keep = "natural_log_exp_and_others"
    return {k: (v if k == keep else set()) for k, v in t.items()}

bacc.get_activation_tables = _single_set_tables

F32 = mybir.dt.float32
F32R = mybir.dt.float32r
BF16 = mybir.dt.bfloat16
AF = mybir.ActivationFunctionType

T = 12
CH = 128
HW = 576
NB = 2          # tracks per core
TP = 9          # output windows
CV = 114        # channels kept per block (96 conv + 18 posenc)
NCONV = 96

# partition tiles of the 576 pixel axis
PT = [(0, 128), (128, 128), (256, 128), (384, 128), (512, 64)]
# free-dim split that respects one-PSUM-bank-per-matmul (512 f32 = 1 bank)
NS = [(0, 512), (512, 64)]

_CACHE = {}


def _posenc() -> np.ndarray:
    ys = np.linspace(-1.0, 1.0, 24)
    xs = np.linspace(-1.0, 1.0, 24)
    g = np.meshgrid(ys, xs, indexing="ij")
    coords = np.stack(g, axis=0)  # (2, 24, 24)
    feats = [coords]
    for i in range(4):
        f = (2.0 ** i) * np.pi * coords
        feats.append(np.sin(f))
        feats.append(np.cos(f))
    pe = np.concatenate(feats, axis=0).astype(np.float32)  # (18, 24, 24)
    return pe.reshape(18, HW)


def _r(ap):
    return ap.bitcast(F32R)


def _build(scale: float) -> bass.Bass:
    nc = bacc.Bacc()
    x_d = nc.declare_dram_parameter("x", [NB, T, CH, HW], F32, isOutput=False)
    wvt_d = nc.declare_dram_parameter("wvt", [CH, NCONV], F32, isOutput=False)
    bv_d = nc.declare_dram_parameter("bvc", [NCONV, 1], F32, isOutput=False)
    pe_d = nc.declare_dram_parameter("pe", [18, HW], F32, isOutput=False)
    id_d = nc.declare_dram_parameter("ident", [128, 128], F32, isOutput=False)
    out_d = nc.declare_dram_parameter("out", [NB, TP, 4 * CV, HW], F32, isOutput=True)

    with tile_mod.TileContext(nc) as tc:
        with (
            nc.allow_low_precision(reason="float32r rounding for PE-rate matmuls"),
            tc.tile_pool(name="const", bufs=1) as cst,
            tc.tile_pool(name="io", bufs=6) as io,
            tc.tile_pool(name="big", bufs=3) as big,
            tc.tile_pool(name="pst", bufs=6) as pst,
            tc.tile_pool(name="stat", bufs=12) as stat,
            tc.tile_pool(name="psB", bufs=3, space=bass.MemorySpace.PSUM) as psB,
            tc.tile_pool(name="psT", bufs=2, space=bass.MemorySpace.PSUM) as psT,
        ):
            wvt = cst.tile([CH, NCONV], F32, tag="wvt")
            nc.sync.dma_start(wvt[:, :], wvt_d[:, :])
            bvc = cst.tile([NCONV, 1], F32, tag="bvc")
            nc.sync.dma_start(bvc[:, :], bv_d[:, :])
            pe_sb = cst.tile([18, HW], F32, tag="pe")
            nc.sync.dma_start(pe_sb[:, :], pe_d[:, :])
            id_sb = cst.tile([128, 128], F32, tag="ident")
            nc.sync.dma_start(id_sb[:, :], id_d[:, :])
            ones_c = cst.tile([CH, 1], BF16, tag="ones_c")
            nc.vector.memset(ones_c[:, :], 1.0)
            ones_r = cst.tile([1, 128], BF16, tag="ones_r")
            nc.vector.memset(ones_r[:, :], 1.0)
            wvt_bf = cst.tile([CH, NCONV], BF16, tag="wvt_bf")
            nc.vector.tensor_copy(wvt_bf[:, :], wvt[:, :])
            ones_rf = cst.tile([1, 128], F32, tag="ones_rf")
            nc.vector.memset(ones_rf[:, :], 1.0)
            ones_rr = cst.tile([1, 128], F32R, tag="ones_rr")
            nc.vector.tensor_copy(ones_rr[:, :], ones_rf[:, :])

            xa_prev_l = [None] * NB
            vT_prev_l = [None] * NB
            s1_l = [None] * NB
            s2_l = [None] * NB
            for t in range(T):
                for b in range(NB):
                    xa_prev = xa_prev_l[b]
                    vT_prev = vT_prev_l[b]
                    s1 = s1_l[b]
                    s2 = s2_l[b]
                    a = t - 1
                    # ---- load x[b, t] and L2-normalize over channels ----
                    xr = io.tile([CH, HW], F32, tag="xraw")
                    nc.sync.dma_start(xr[:, :], x_d[b, t, :, :])
                    xsq = io.tile([CH, HW], BF16, tag="xsq")
                    nc.scalar.activation(xsq[:, :], xr[:, :], AF.Square)
                    xb = io.tile([CH, HW], BF16, tag="xb")
                    nc.vector.tensor_copy(xb[:, :], xr[:, :])
                    ssp = psB.tile([128, HW], F32, tag="ps")
                    for (o, w) in NS:
                        nc.tensor.matmul(
                            ssp[0:1, o:o + w], ones_c[:, :], xsq[:, o:o + w],
                            start=True, stop=True,
                        )
                    lns = stat.tile([1, HW], F32, tag="lns")
                    nc.scalar.activation(lns[:, :], ssp[0:1, :], AF.Ln)
                    inv = stat.tile([1, HW], BF16, tag="inv")
                    nc.scalar.activation(inv[:, :], lns[:, :], AF.Exp, scale=-0.5)
                    bcp = psB.tile([128, HW], F32, tag="ps")
                    for (o, w) in NS:
                        nc.tensor.matmul(
                            bcp[:, o:o + w], ones_r[:, :], inv[:, o:o + w],
                            start=True, stop=True,
                        )
                    xa = io.tile([CH, HW], BF16, tag="xatt")
                    nc.vector.tensor_mul(xa[:, :], xr[:, :], bcp[:, :])

                    # ---- V_t = [Wv[32:] @ x + bv[32:]; posenc]  (114 x 576) ----
                    vps = psB.tile([128, HW], F32, tag="ps")
                    for (o, w) in NS:
                        nc.tensor.matmul(
                            vps[0:NCONV, o:o + w], wvt_bf[:, :], xb[:, o:o + w],
                            start=True, stop=True,
                        )
                    v_sb = io.tile([CV, HW], F32, tag="v")
                    nc.scalar.activation(
                        v_sb[0:NCONV, :], vps[0:NCONV, :], AF.Identity,
                        bias=bvc[:, 0:1],
                    )
                    nc.vector.tensor_copy(v_sb[NCONV:CV, :], pe_sb[:, :])
                    if t >= 3:
                        nc.sync.dma_start(out_d[b, t - 3, 0:CV, :], v_sb[:, :])

                    # ---- V^T tiles [pixel, channel] for the P1 product ----
                    vT = io.tile([128, 5, CV], BF16, tag="vT")
                    for i, (po, pw) in enumerate(PT):
                        tp = psT.tile([128, 128], F32, tag="tp")
                        nc.tensor.transpose(
                            tp[0:pw, 0:CV], v_sb[:, po:po + pw], id_sb[0:CV, 0:CV]
                        )
                        if i % 2 == 0:
                            nc.scalar.copy(vT[0:pw, i, :], tp[0:pw, 0:CV])
                        else:
                            nc.vector.tensor_copy(vT[0:pw, i, :], tp[0:pw, 0:CV])

                    if t >= 1:
                        # ---- A_a^T directly: E^T = keys^T queries, exp, no
                        # per-row transpose.  Normalization by 1/Z folds into
                        # the product PSUM->SBUF copies. ----
                        aT = big.tile([128, 5, HW], BF16, tag="aT")
                        for mi, (mo, mw) in enumerate(PT):
                            ets = psB.tile([128, HW], F32, tag="ps")
                            for (o, w) in NS:
                                nc.tensor.matmul(
                                    ets[0:mw, o:o + w],
                                    xa_prev[:, mo:mo + mw],
                                    xa[:, o:o + w],
                                    start=True, stop=True,
                                )
                            nc.scalar.activation(
                                aT[0:mw, mi, :], ets[0:mw, :], AF.Exp, scale=scale
                            )
                        # Z_n = sum_m exp(E^T[m, n]) via ones-matmul over partitions
                        zp = psB.tile([128, HW], F32, tag="ps")
                        for (o, w) in NS:
                            for ki, (ko, kw) in enumerate(PT):
                                nc.tensor.matmul(
                                    zp[0:1, o:o + w],
                                    ones_c[0:kw, :],
                                    aT[0:kw, ki, o:o + w],
                                    start=(ki == 0), stop=(ki == 4),
                                )
                        izr = stat.tile([1, HW], F32R, tag="izr")
                        nc.vector.reciprocal(izr[:, :], zp[0:1, :])
                        izp = psB.tile([128, HW], F32, tag="ps")
                        for (o, w) in NS:
                            nc.tensor.matmul(
                                izp[:, o:o + w], ones_rr[:, :], izr[:, o:o + w],
                                start=True, stop=True,
                            )
                        izb = big.tile([CV, HW], F32, tag="izb")
                        nc.vector.tensor_copy(izb[:, :], izp[0:CV, :])

                        # ---- chain products (unnormalized; scaled by iZ on copy-out) ----
                        def product(lhsT_tiles):
                            pp = psB.tile([128, HW], F32, tag="ps")
                            for (o, w) in NS:
                                for ki, (ko, kw) in enumerate(PT):
                                    nc.tensor.matmul(
                                        pp[0:CV, o:o + w],
                                        lhsT_tiles[0:kw, ki, :],
                                        aT[0:kw, ki, o:o + w],
                                        start=(ki == 0), stop=(ki == 4),
                                    )
                            return pp

                        def to_sbuf(pp):
                            sb = pst.tile([CV, HW], F32, tag="p")
                            nc.vector.tensor_mul(sb[:, :], pp[0:CV, :], izb[:, :])
                            return sb

                        def transpose_state(sb):
                            st = pst.tile([128, 5, CV], BF16, tag="s")
                            for i, (po, pw) in enumerate(PT):
                                tp = psT.tile([128, 128], F32, tag="tp")
                                nc.tensor.transpose(
                                    tp[0:pw, 0:CV], sb[:, po:po + pw],
                                    id_sb[0:CV, 0:CV],
                                )
                                if i % 2 == 0:
                                    nc.vector.tensor_copy(st[0:pw, i, :], tp[0:pw, 0:CV])
                                else:
                                    nc.scalar.copy(st[0:pw, i, :], tp[0:pw, 0:CV])
                            return st

                        p1 = to_sbuf(product(vT_prev))
                        if a >= 2:
                            nc.sync.dma_start(out_d[b, a - 2, CV:2 * CV, :], p1[:, :])
                        p2 = None
                        if s1 is not None:
                            p2 = to_sbuf(product(s1))
                            if a >= 2:
                                nc.sync.dma_start(
                                    out_d[b, a - 2, 2 * CV:3 * CV, :], p2[:, :]
                                )
                        if s2 is not None:
                            p3 = to_sbuf(product(s2))
                            nc.sync.dma_start(
                                out_d[b, a - 2, 3 * CV:4 * CV, :], p3[:, :]
                            )
                        if a < 10:
                            s1_l[b] = transpose_state(p1)
                            if p2 is not None:
                                s2_l[b] = transpose_state(p2)

                    xa_prev_l[b] = xa
                    vT_prev_l[b] = vT
    nc.compile()
    return nc


def _get_nc(scale: float) -> bass.Bass:
    key = round(scale, 12)
    if key not in _CACHE:
        _CACHE[key] = _build(scale)
    return _CACHE[key]


def kernel(x, Wv, bv, temp):
    x = np.asarray(x, dtype=np.float32)
    Wv = np.asarray(Wv, dtype=np.float32)
    bv = np.asarray(bv, dtype=np.float32)
    bs, N, T_, ch, h, w = x.shape
    BN = bs * N
    xf = np.ascontiguousarray(x.reshape(BN, T_, ch, h * w))

    scale = float(ch) ** (-0.5) / float(np.asarray(temp))
    nc = _get_nc(scale)

    wvt = np.ascontiguousarray(Wv[32:, :].T)          # (128, 96)
    bvc = np.ascontiguousarray(bv[32:].reshape(NCONV, 1))
    pe = _posenc()
    ident = np.eye(128, dtype=np.float32)

    in_maps = []
    for c in range(8):
        in_maps.append({
            "x": np.ascontiguousarray(xf[c * NB:(c + 1) * NB]),
            "wvt": wvt,
            "bvc": bvc,
            "pe": pe,
            "ident": ident,
        })
    res = run_bass_kernel_spmd(nc, in_maps, core_ids=list(range(8)))
    outs = [res.results[c]["out"] for c in range(8)]
    return np.concatenate(outs, axis=0).astype(np.float32)
